# revision 20
# baseline (speedup 1.0000x reference)
"""Sliding-window attention (w=11) Trainium2 Bass kernel — v2.

Problem: x:(2048,4,1024) f32; q/k/v = x @ W{q,k,v}.T ; per (l,b,head):
  energy[w] = q . (k[l+w-5] + pe[:,w]),  attn = softmax(energy/32),
  out = sum_w attn[w] * v[l+w-5].

Sharding: sequence-parallel over l across 8 cores (256 l each, halo 5,
zero-padded at global edges). Weights/pe replicated; SPMD, no collectives.

v2 design (vs v1 baseline @538us):
  - all high-frequency DMAs issued from the GpSimd (Pool) queue: SW-DGE
    dispatch is ~25ns of queue time vs 565ns on sync HW-DGE.
  - per (tck,b,wave-of-4-heads): ONE exp, ONE spill, ONE batched band
    gather, ONE batched attn scatter, ONE reload, ONE output store.
  - pe-term matmuls merged 2-heads-at-a-time via block-diagonal pe rhs.
  - tail (10-row) transposes of 4 heads done as ONE strided-AP transpose;
    tail AV as ONE block-diagonal matmul.
  - LDWEIGHTS-friendly projection order: each stationary feeds 2-3
    consecutive matmuls.
  - software-pipelined attention loop (skew 2) so PE never waits on the
    DRAM band roundtrip.
  - outputs stored bf16, upcast on host.
"""
import os
import sys

sys.path.insert(0, "/opt/trn_rl_repo")

from contextlib import ExitStack

import numpy as np
import ml_dtypes

import concourse.bass as bass
import concourse.mybir as mybir
import concourse.tile as tile
from concourse import bacc
from concourse import bass_utils

BF16 = mybir.dt.bfloat16
F32 = mybir.dt.float32
NPBF16 = ml_dtypes.bfloat16

L, B, C = 2048, 4, 1024
H, D, W = 16, 64, 11
PAD = 5
NCORES = 8
LSH = L // NCORES            # 256 central l per core
LLOC = LSH + 2 * PAD         # 266 l rows incl halo
R = LLOC * B                 # 1064 rows
RC = LSH * B                 # 1024 central rows

ODD64 = int(os.environ.get("K2_ODD64", "0"))   # base-64 matmul operands CRASH the device; keep 0
BC0 = int(os.environ.get("K2_BC0", "1"))       # stride-0 rden broadcast
SKEW = int(os.environ.get("K2_SKEW", "3"))     # attention software-pipeline depth

NW = 2 * B * 4               # 32 waves: (tck, b, wv4)
AFD_N = 8                    # rotating banded-attn DRAM buffers

if int(os.environ.get("K2_LDWOPT", "0")):
    # opt-in experiment: let walrus dedupe LDWEIGHTS for repeated stationaries
    _orig_run_command = bass_utils.run_command

    def _run_command_ldwopt(argv, **kwargs):
        argv = ["--enable-ldw-opt=true" if a == "--enable-ldw-opt=false" else a
                for a in argv]
        return _orig_run_command(argv, **kwargs)

    bass_utils.run_command = _run_command_ldwopt

_CACHED = {}


def _build_nc():
    if "nc" in _CACHED:
        return _CACHED["nc"]
    nc = bacc.Bacc(None, target_bir_lowering=False)

    # ---- DRAM I/O ----
    xT_d = nc.dram_tensor("xT8", [128, 8 * R], BF16, kind="ExternalInput")
    wq_d = nc.dram_tensor("wq8", [128, 8 * C], BF16, kind="ExternalInput")
    wk_d = nc.dram_tensor("wk8", [128, 8 * C], BF16, kind="ExternalInput")
    wv_d = nc.dram_tensor("wv8", [128, 8 * C], BF16, kind="ExternalInput")
    pe_d = nc.dram_tensor("pe2bd", [128, 8 * 22], BF16, kind="ExternalInput")
    id_d = nc.dram_tensor("ident", [128, 128], BF16, kind="ExternalInput")
    y_d = nc.dram_tensor("y", [RC, C], BF16, kind="ExternalOutput")
    # internal DRAM: per-wave exp(E) spill; rotating banded-attn buffers
    ed = [nc.dram_tensor(f"ed{i}", [128, 640], BF16, kind="Internal")
          for i in range(NW)]
    afd = [nc.dram_tensor(f"afd{i}", [128, 644], BF16, kind="Internal")
           for i in range(AFD_N)]

    with ExitStack() as ctx:
        _ctr = [0]

        def sb(shape, dt, nm):
            _ctr[0] += 1
            return ctx.enter_context(
                nc.sbuf_tensor(f"{nm}_{_ctr[0]}", shape, dt))

        # ---- static SBUF ----
        xT8 = sb([128, 8 * R], BF16, "sx")
        wq8 = sb([128, 8 * C], BF16, "swq")
        wk8 = sb([128, 8 * C], BF16, "swk")
        wv8 = sb([128, 8 * C], BF16, "swv")
        qT = [sb([128, RC], BF16, "sq") for _ in range(8)]
        kbd = [sb([128, 2 * R], BF16, "skb") for _ in range(8)]
        vfull = [[sb([128, C], BF16, "svf") for _ in range(2)] for _ in range(B)]
        vtail = [sb([10, C], BF16, "svt") for _ in range(B)]
        pe2 = sb([128, 8 * 22], BF16, "spe")
        ident = sb([128, 128], BF16, "sid")
        zaf = sb([128, 644], BF16, "szf")

        with tile.TileContext(nc) as tc:
            # ---- init loads (gpsimd queue: ~25ns dispatch each) ----
            # x and wq split in column-halves so the first q matmuls start early
            for h0, hn in ((0, 266), (266, 266)):
                nc.gpsimd.dma_start(
                    xT8[:].rearrange("p (k r) -> p k r", k=8)[:, :, h0:h0 + hn],
                    bass.AP(xT_d, h0, [[8 * R, 128], [R, 8], [1, hn]]))
            for h0 in (0, 256):
                nc.gpsimd.dma_start(
                    wq8[:].rearrange("p (k c) -> p k c", k=8)[:, :, h0:h0 + 256],
                    bass.AP(wq_d, h0, [[8 * C, 128], [C, 8], [1, 256]]))
            nc.gpsimd.dma_start(
                xT8[:].rearrange("p (k r) -> p k r", k=8)[:, :, 532:R],
                bass.AP(xT_d, 532, [[8 * R, 128], [R, 8], [1, R - 532]]))
            nc.gpsimd.dma_start(
                wq8[:].rearrange("p (k c) -> p k c", k=8)[:, :, 512:C],
                bass.AP(wq_d, 512, [[8 * C, 128], [C, 8], [1, 512]]))
            nc.gpsimd.dma_start(wk8[:, :], wk_d[:])
            nc.gpsimd.dma_start(wv8[:, :], wv_d[:])
            nc.gpsimd.dma_start(pe2[:, :], pe_d[:])
            nc.gpsimd.dma_start(ident[:, :], id_d[:])
            nc.vector.memset(zaf[:, :], 0.0)
            for ct in range(8):
                nc.vector.memset(kbd[ct][64:128, 0:R], 0.0)
                nc.vector.memset(kbd[ct][0:64, R:2 * R], 0.0)
            for j in range(AFD_N):
                nc.gpsimd.dma_start(afd[j][:], zaf[:, :])

            xk = xT8[:].rearrange("p (k r) -> p k r", k=8)

            # ---- projections ----
            dr = [0]

            def drain(dst, src):
                # rotate psum->sbuf drains between ACT and DVE
                dr[0] += 1
                if dr[0] % 2:
                    nc.scalar.copy(dst, src)
                else:
                    nc.vector.tensor_copy(dst, src)

            with tc.tile_pool(name="pp", bufs=6, space="PSUM") as pp, \
                 tc.tile_pool(name="ppt", bufs=2, space="PSUM") as ppt:
                for ct in range(8):
                    # q: cols r=20..1044 (central), stationary reused 2x
                    ps0 = pp.tile([128, 512], F32, tag="ps")
                    ps1 = pp.tile([128, 512], F32, tag="ps")
                    for ki in range(8):
                        st = wq8[:, C * ki + 128 * ct: C * ki + 128 * ct + 128]
                        nc.tensor.matmul(ps0[:, :], st, xk[:, ki, 20:532],
                                         start=(ki == 0), stop=(ki == 7))
                        nc.tensor.matmul(ps1[:, :], st, xk[:, ki, 532:1044],
                                         start=(ki == 0), stop=(ki == 7))
                    drain(qT[ct][:, 0:512], ps0[:, :])
                    drain(qT[ct][:, 512:1024], ps1[:, :])
                    # k: full 1064 cols, stationary reused 3x
                    ps2 = pp.tile([128, 512], F32, tag="ps")
                    ps3 = pp.tile([128, 512], F32, tag="ps")
                    ps4 = ppt.tile([128, 40], F32, tag="pst")
                    for ki in range(8):
                        st = wk8[:, C * ki + 128 * ct: C * ki + 128 * ct + 128]
                        nc.tensor.matmul(ps2[:, :], st, xk[:, ki, 0:512],
                                         start=(ki == 0), stop=(ki == 7))
                        nc.tensor.matmul(ps3[:, :], st, xk[:, ki, 512:1024],
                                         start=(ki == 0), stop=(ki == 7))
                        nc.tensor.matmul(ps4[:, :], st, xk[:, ki, 1024:1064],
                                         start=(ki == 0), stop=(ki == 7))
                    for c0, cn, pst in ((0, 512, ps2), (512, 512, ps3),
                                        (1024, 40, ps4)):
                        drain(kbd[ct][0:64, c0:c0 + cn], pst[0:64, 0:cn])
                        drain(kbd[ct][64:128, R + c0:R + c0 + cn],
                              pst[64:128, 0:cn])
                # v row-major per (b, ltile); stationary (x-slice) reused 2x
                for b in range(B):
                    for t in range(3):
                        rows = 128 if t < 2 else 10
                        ps0 = pp.tile([128, 512], F32, tag="ps")
                        ps1 = pp.tile([128, 512], F32, tag="ps")
                        for ki in range(8):
                            lhs = (xk[:, ki, :]
                                   .rearrange("p (l four) -> p l four", four=4)
                                   [:, 128 * t:128 * t + rows, b])
                            nc.tensor.matmul(ps0[0:rows, :], lhs,
                                             wv8[:, C * ki: C * ki + 512],
                                             start=(ki == 0), stop=(ki == 7))
                            nc.tensor.matmul(ps1[0:rows, :], lhs,
                                             wv8[:, C * ki + 512: C * ki + 1024],
                                             start=(ki == 0), stop=(ki == 7))
                        dst = vfull[b][t] if t < 2 else vtail[b]
                        drain(dst[0:rows, 0:512], ps0[0:rows, :])
                        drain(dst[0:rows, 512:1024], ps1[0:rows, :])

            # ---- attention: software-pipelined waves ----
            waves = [(tck, b, wv4)
                     for tck in range(2) for b in range(B) for wv4 in range(4)]

            with tc.tile_pool(name="ep", bufs=2, space="PSUM") as ep, \
                 tc.tile_pool(name="tp", bufs=2, space="PSUM") as tp, \
                 tc.tile_pool(name="op", bufs=2, space="PSUM") as op, \
                 tc.tile_pool(name="asb", bufs=6) as asb, \
                 tc.tile_pool(name="bsb", bufs=7) as bsb, \
                 tc.tile_pool(name="csb", bufs=5) as csb, \
                 tc.tile_pool(name="ysb", bufs=3) as ysb:

                state = {}

                def stage_a(wi):
                    tck, b, wv4 = waves[wi]
                    Ew = ep.tile([128, 1024], F32, tag="ew")
                    # merged 2-head E matmuls via block-diagonal kbd
                    for half in range(2):
                        ct = 2 * wv4 + half
                        lhs = (qT[ct][:]
                               .rearrange("p (l four) -> p l four", four=4)
                               [:, 128 * tck:128 * tck + 128, b])
                        rhs = bass.AP(kbd[ct], 4 * 128 * tck + b,
                                      [[2 * R, 128], [R, 2], [4, 138]])
                        out = bass.AP(Ew.tensor, Ew.offset + 512 * half,
                                      [[1024, 128], [160, 2], [1, 138]])
                        nc.tensor.matmul(out, lhs, rhs, start=True, stop=True)
                        nc.tensor.matmul(Ew[:, 512 * half + 298:512 * half + 320],
                                         lhs, pe2[:, 22 * ct:22 * ct + 22],
                                         start=True, stop=True)
                    # one exp over both halves (E and P regions)
                    ex = asb.tile([128, 640], BF16, tag="ex")
                    nc.scalar.activation(
                        ex[:].rearrange("p (a c) -> p a c", a=2)[:, :, 0:320],
                        Ew[:].rearrange("p (a c) -> p a c", a=2)[:, :, 0:320],
                        mybir.ActivationFunctionType.Exp,
                    )
                    edw = ed[wi]
                    nc.sync.dma_start(edw[:], ex[:, :])
                    # single batched diagonal band gather: heads at stride 160
                    bnd = bsb.tile([128, 44], BF16, tag="bnd")
                    nc.gpsimd.dma_start(
                        bnd[:].rearrange("p (h w) -> p h w", w=W),
                        bass.AP(edw, 0, [[641, 128], [160, 4], [1, W]]),
                    )
                    # numerators = band_exp * exp(P); denominator; normalize
                    exP = bass.AP(ex.tensor, ex.offset + 298,
                                  [[640, 128], [320, 2], [11, 2], [1, W]])
                    t1 = bsb.tile([128, 44], F32, tag="t1")
                    nc.vector.tensor_mul(
                        t1[:].rearrange("p (a q w) -> p a q w", a=2, q=2),
                        bnd[:].rearrange("p (a q w) -> p a q w", a=2, q=2),
                        exP,
                    )
                    den = bsb.tile([128, 4], F32, tag="den")
                    nc.vector.tensor_reduce(
                        den[:, :],
                        t1[:].rearrange("p (h w) -> p h w", w=W),
                        axis=mybir.AxisListType.X,
                        op=mybir.AluOpType.add,
                    )
                    rden = bsb.tile([128, 4], F32, tag="rden")
                    nc.vector.reciprocal(rden[:, :], den[:, :])
                    att = bsb.tile([128, 44], BF16, tag="att")
                    if BC0:
                        rbc = bass.AP(rden.tensor, rden.offset,
                                      [[4, 128], [1, 4], [0, W]])
                        nc.vector.tensor_mul(
                            att[:].rearrange("p (h w) -> p h w", w=W),
                            t1[:].rearrange("p (h w) -> p h w", w=W),
                            rbc,
                        )
                    else:
                        for hh in range(4):
                            nc.vector.tensor_scalar_mul(
                                att[:, W * hh:W * hh + W],
                                t1[:, W * hh:W * hh + W],
                                rden[:, hh:hh + 1])
                    # batched diagonal scatter into banded buffer + reload
                    afj = afd[wi % AFD_N]
                    nc.gpsimd.dma_start(
                        bass.AP(afj, 0, [[645, 128], [161, 4], [1, W]]),
                        att[:].rearrange("p (h w) -> p h w", w=W),
                    )
                    af4 = csb.tile([128, 644], BF16, tag="af4")
                    nc.sync.dma_start(af4[:, :], afj[:])
                    state[wi] = af4

                def stage_t(wi):
                    # transposes: 4 mains [128,128] + 4 tails [128,10]->[10,128]
                    af4 = state.pop(wi)
                    tps = tp.tile([128, 1024], BF16, tag="tps")
                    for hh in range(4):
                        nc.tensor.transpose(
                            tps[:, 128 * hh:128 * hh + 128],
                            af4[:, 161 * hh:161 * hh + 128], ident[:, :])
                        nc.tensor.transpose(
                            tps[0:10, 512 + 128 * hh:640 + 128 * hh],
                            af4[:, 161 * hh + 128:161 * hh + 138], ident[:, :])
                    afT = csb.tile([128, 1024], BF16, tag="afT")
                    nc.vector.tensor_copy(afT[:, 0:512], tps[:, 0:512])
                    nc.scalar.copy(afT[0:10, 512:1024], tps[0:10, 512:1024])
                    state[("afT", wi)] = afT

                def stage_b(wi):
                    tck, b, wv4 = waves[wi]
                    afT = state.pop(("afT", wi))
                    vsrc = vfull[b][1] if tck == 0 else vtail[b]
                    # AV: 4 mains (K=128) + 4 tails (K=10)
                    ops = op.tile([128, 256], F32, tag="ops")
                    for hh in range(4):
                        h = 4 * wv4 + hh
                        nc.tensor.matmul(
                            ops[:, 64 * hh:64 * hh + 64],
                            afT[:, 128 * hh:128 * hh + 128],
                            vfull[b][tck][:, 64 * h:64 * h + 64],
                            start=True, stop=False, skip_group_check=True)
                        nc.tensor.matmul(
                            ops[:, 64 * hh:64 * hh + 64],
                            afT[0:10, 512 + 128 * hh:640 + 128 * hh],
                            vsrc[0:10, 64 * h:64 * h + 64],
                            start=False, stop=True, skip_group_check=True)
                    # accumulate 4 waves into one [128,1024] tile, store once
                    if wv4 == 0:
                        ybt = ysb.tile([128, 1024], BF16, tag="yb")
                        state["yb"] = ybt
                    yb = state["yb"]
                    if (wi % 2) == 0:
                        nc.vector.tensor_copy(
                            yb[:, 256 * wv4:256 * wv4 + 256], ops[:, :])
                    else:
                        nc.scalar.copy(
                            yb[:, 256 * wv4:256 * wv4 + 256], ops[:, :])
                    if wv4 == 3:
                        nc.sync.dma_start(
                            bass.AP(y_d, (512 * tck + b) * C,
                                    [[4 * C, 128], [1, C]]),
                            yb[:, :],
                        )

                # A leads by SKEW, T (transposes) leads AV by 1 so the
                # psum->sbuf drain never stalls the in-order PE queue
                for wi in range(min(SKEW, NW)):
                    stage_a(wi)
                stage_t(0)
                for wi in range(NW):
                    if wi + SKEW < NW:
                        stage_a(wi + SKEW)
                    if wi + 1 < NW:
                        stage_t(wi + 1)
                    stage_b(wi)

    nc.compile()
    _CACHED["nc"] = nc
    return nc


def host_prep(x, Wq, Wk, Wv, pe):
    """Build per-core input maps (host-side shard + layout prep)."""
    if "host" in _CACHED:
        return _CACHED["host"]
    x = np.asarray(x, np.float32)
    xp = np.zeros((L + 2 * PAD, B, C), np.float32)
    xp[PAD:PAD + L] = x

    def w8(Wt):
        # [c_in, c_out] -> [128, (ki, c_out)]
        a = np.asarray(Wt, np.float32).reshape(8, 128, C).transpose(1, 0, 2)
        return np.ascontiguousarray(a.reshape(128, 8 * C)).astype(NPBF16)

    wq8 = w8(np.asarray(Wq, np.float32).T / 32.0)
    wk8 = w8(np.asarray(Wk, np.float32).T)
    wv8 = w8(np.asarray(Wv, np.float32).T)
    pe = np.asarray(pe, np.float32)
    # block-diagonal pe pairs: [128, (ct, 22)]
    pebd = np.zeros((128, 8, 22), np.float32)
    for ct in range(8):
        pebd[0:64, ct, 0:11] = pe[2 * ct]
        pebd[64:128, ct, 11:22] = pe[2 * ct + 1]
    pebd = np.ascontiguousarray(pebd.reshape(128, 8 * 22)).astype(NPBF16)
    ident = np.eye(128, dtype=NPBF16)
    in_maps = []
    for c in range(NCORES):
        xs = xp[LSH * c:LSH * c + LLOC].reshape(R, C)
        x8 = np.ascontiguousarray(
            xs.T.reshape(8, 128, R).transpose(1, 0, 2).reshape(128, 8 * R)
        ).astype(NPBF16)
        in_maps.append({
            "xT8": x8, "wq8": wq8, "wk8": wk8, "wv8": wv8,
            "pe2bd": pebd, "ident": ident,
        })
    _CACHED["host"] = in_maps
    return in_maps


LAST_RES = [None]


def kernel(x, Wq, Wk, Wv, pe, _want_time=False):
    nc = _build_nc()
    in_maps = host_prep(x, Wq, Wk, Wv, pe)
    kw = {}
    if _want_time:
        kw = dict(trace=True)
    res = bass_utils.run_bass_kernel_spmd(
        nc, in_maps, core_ids=list(range(NCORES)), **kw)
    LAST_RES[0] = res
    y = np.concatenate(
        [np.asarray(r["y"]).astype(np.float32) for r in res.results], axis=0)
    out = y.reshape(L, B, C)
    if _want_time:
        return out, res.exec_time_ns
    return out


# revision 21
# speedup vs baseline: 1.0076x; 1.0076x over previous
"""Sliding-window attention (w=11) Trainium2 Bass kernel — v2.

Problem: x:(2048,4,1024) f32; q/k/v = x @ W{q,k,v}.T ; per (l,b,head):
  energy[w] = q . (k[l+w-5] + pe[:,w]),  attn = softmax(energy/32),
  out = sum_w attn[w] * v[l+w-5].

Sharding: sequence-parallel over l across 8 cores (256 l each, halo 5,
zero-padded at global edges). Weights/pe replicated; SPMD, no collectives.

v2 design (vs v1 baseline @538us):
  - all high-frequency DMAs issued from the GpSimd (Pool) queue: SW-DGE
    dispatch is ~25ns of queue time vs 565ns on sync HW-DGE.
  - per (tck,b,wave-of-4-heads): ONE exp, ONE spill, ONE batched band
    gather, ONE batched attn scatter, ONE reload, ONE output store.
  - pe-term matmuls merged 2-heads-at-a-time via block-diagonal pe rhs.
  - tail (10-row) transposes of 4 heads done as ONE strided-AP transpose;
    tail AV as ONE block-diagonal matmul.
  - LDWEIGHTS-friendly projection order: each stationary feeds 2-3
    consecutive matmuls.
  - software-pipelined attention loop (skew 2) so PE never waits on the
    DRAM band roundtrip.
  - outputs stored bf16, upcast on host.
"""
import os
import sys

sys.path.insert(0, "/opt/trn_rl_repo")

from contextlib import ExitStack

import numpy as np
import ml_dtypes

import concourse.bass as bass
import concourse.mybir as mybir
import concourse.tile as tile
from concourse import bacc
from concourse import bass_utils

BF16 = mybir.dt.bfloat16
F32 = mybir.dt.float32
NPBF16 = ml_dtypes.bfloat16

L, B, C = 2048, 4, 1024
H, D, W = 16, 64, 11
PAD = 5
NCORES = 8
LSH = L // NCORES            # 256 central l per core
LLOC = LSH + 2 * PAD         # 266 l rows incl halo
R = LLOC * B                 # 1064 rows
RC = LSH * B                 # 1024 central rows

ODD64 = int(os.environ.get("K2_ODD64", "0"))   # base-64 matmul operands CRASH the device; keep 0
BC0 = int(os.environ.get("K2_BC0", "1"))       # stride-0 rden broadcast
SKEW = int(os.environ.get("K2_SKEW", "3"))     # attention software-pipeline depth

NW = 2 * B * 4               # 32 waves: (tck, b, wv4)
AFD_N = 8                    # rotating banded-attn DRAM buffers

if int(os.environ.get("K2_LDWOPT", "0")):
    # opt-in experiment: let walrus dedupe LDWEIGHTS for repeated stationaries
    _orig_run_command = bass_utils.run_command

    def _run_command_ldwopt(argv, **kwargs):
        argv = ["--enable-ldw-opt=true" if a == "--enable-ldw-opt=false" else a
                for a in argv]
        return _orig_run_command(argv, **kwargs)

    bass_utils.run_command = _run_command_ldwopt

_CACHED = {}


def _build_nc():
    if "nc" in _CACHED:
        return _CACHED["nc"]
    nc = bacc.Bacc(None, target_bir_lowering=False)

    # ---- DRAM I/O ----
    xT_d = nc.dram_tensor("xT8", [128, 8 * R], BF16, kind="ExternalInput")
    wq_d = nc.dram_tensor("wq8", [128, 8 * C], BF16, kind="ExternalInput")
    wk_d = nc.dram_tensor("wk8", [128, 8 * C], BF16, kind="ExternalInput")
    wv_d = nc.dram_tensor("wv8", [128, 8 * C], BF16, kind="ExternalInput")
    pe_d = nc.dram_tensor("pe2bd", [128, 8 * 22], BF16, kind="ExternalInput")
    id_d = nc.dram_tensor("ident", [128, 128], BF16, kind="ExternalInput")
    y_d = nc.dram_tensor("y", [RC, C], BF16, kind="ExternalOutput")
    # internal DRAM: per-wave exp(E) spill; rotating banded-attn buffers
    ed = [nc.dram_tensor(f"ed{i}", [128, 640], BF16, kind="Internal")
          for i in range(NW)]
    afd = [nc.dram_tensor(f"afd{i}", [128, 644], BF16, kind="Internal")
           for i in range(AFD_N)]

    with ExitStack() as ctx:
        _ctr = [0]

        def sb(shape, dt, nm):
            _ctr[0] += 1
            return ctx.enter_context(
                nc.sbuf_tensor(f"{nm}_{_ctr[0]}", shape, dt))

        # ---- static SBUF ----
        xT8 = sb([128, 8 * R], BF16, "sx")
        wq8 = sb([128, 8 * C], BF16, "swq")
        wk8 = sb([128, 8 * C], BF16, "swk")
        wv8 = sb([128, 8 * C], BF16, "swv")
        qT = [sb([128, RC], BF16, "sq") for _ in range(8)]
        kbd = [sb([128, 2 * R], BF16, "skb") for _ in range(8)]
        vfull = [[sb([128, C], BF16, "svf") for _ in range(2)] for _ in range(B)]
        vtail = [sb([10, C], BF16, "svt") for _ in range(B)]
        pe2 = sb([128, 8 * 22], BF16, "spe")
        ident = sb([128, 128], BF16, "sid")
        zaf = sb([128, 644], BF16, "szf")

        with tile.TileContext(nc) as tc:
            # ---- init loads (gpsimd queue: ~25ns dispatch each) ----
            # x and wq split in column-halves so the first q matmuls start early
            for h0, hn in ((0, 266), (266, 266), (532, 266), (798, 266)):
                nc.gpsimd.dma_start(
                    xT8[:].rearrange("p (k r) -> p k r", k=8)[:, :, h0:h0 + hn],
                    bass.AP(xT_d, h0, [[8 * R, 128], [R, 8], [1, hn]]))
            for h0 in (0, 512):
                nc.gpsimd.dma_start(
                    wv8[:].rearrange("p (k c) -> p k c", k=8)[:, :, h0:h0 + 512],
                    bass.AP(wv_d, h0, [[8 * C, 128], [C, 8], [1, 512]]))
            nc.gpsimd.dma_start(wq8[:, :], wq_d[:])
            nc.gpsimd.dma_start(wk8[:, :], wk_d[:])
            nc.gpsimd.dma_start(pe2[:, :], pe_d[:])
            nc.gpsimd.dma_start(ident[:, :], id_d[:])
            nc.gpsimd.memset(zaf[:, :], 0.0)
            for ct in range(8):
                nc.gpsimd.memset(kbd[ct][64:128, 0:R], 0.0)
                nc.gpsimd.memset(kbd[ct][0:64, R:2 * R], 0.0)
            for j in range(AFD_N):
                nc.gpsimd.dma_start(afd[j][:], zaf[:, :])

            xk = xT8[:].rearrange("p (k r) -> p k r", k=8)

            # ---- projections ----
            dr = [0]

            def drain(dst, src):
                # rotate psum->sbuf drains between ACT and DVE
                dr[0] += 1
                if dr[0] % 2:
                    nc.scalar.copy(dst, src)
                else:
                    nc.vector.tensor_copy(dst, src)

            with tc.tile_pool(name="ppv", bufs=6, space="PSUM") as ppv:
                # v row-major per (b, ltile); stationary (x-slice) reused 2x
                for b in range(B):
                    for t in range(3):
                        rows = 128 if t < 2 else 10
                        ps0 = ppv.tile([128, 512], F32, tag="ps")
                        ps1 = ppv.tile([128, 512], F32, tag="ps")
                        for ki in range(8):
                            lhs = (xk[:, ki, :]
                                   .rearrange("p (l four) -> p l four", four=4)
                                   [:, 128 * t:128 * t + rows, b])
                            nc.tensor.matmul(ps0[0:rows, :], lhs,
                                             wv8[:, C * ki: C * ki + 512],
                                             start=(ki == 0), stop=(ki == 7))
                            nc.tensor.matmul(ps1[0:rows, :], lhs,
                                             wv8[:, C * ki + 512: C * ki + 1024],
                                             start=(ki == 0), stop=(ki == 7))
                        dst = vfull[b][t] if t < 2 else vtail[b]
                        drain(dst[0:rows, 0:512], ps0[0:rows, :])
                        drain(dst[0:rows, 512:1024], ps1[0:rows, :])

            # ---- attention waves (wv4-major) overlapped with q/k proj ----
            waves = [(tck, b, wv4)
                     for wv4 in range(4) for tck in range(2) for b in range(B)]

            with tc.tile_pool(name="ep", bufs=1, space="PSUM") as ep, \
                 tc.tile_pool(name="tp", bufs=1, space="PSUM") as tp, \
                 tc.tile_pool(name="op", bufs=2, space="PSUM") as op, \
                 tc.tile_pool(name="ppqk", bufs=3, space="PSUM") as ppqk, \
                 tc.tile_pool(name="asb", bufs=6) as asb, \
                 tc.tile_pool(name="bsb", bufs=7) as bsb, \
                 tc.tile_pool(name="csb", bufs=5) as csb, \
                 tc.tile_pool(name="ysb", bufs=3) as ysb:

                state = {}

                def qk(ct):
                    # q: cols r=20..1044 (central), stationary reused 2x
                    ps0 = ppqk.tile([128, 512], F32, tag="ps")
                    ps1 = ppqk.tile([128, 512], F32, tag="ps")
                    for ki in range(8):
                        st = wq8[:, C * ki + 128 * ct: C * ki + 128 * ct + 128]
                        nc.tensor.matmul(ps0[:, :], st, xk[:, ki, 20:532],
                                         start=(ki == 0), stop=(ki == 7))
                        nc.tensor.matmul(ps1[:, :], st, xk[:, ki, 532:1044],
                                         start=(ki == 0), stop=(ki == 7))
                    drain(qT[ct][:, 0:512], ps0[:, :])
                    drain(qT[ct][:, 512:1024], ps1[:, :])
                    # k: full 1064 cols, stationary reused 3x
                    ps2 = ppqk.tile([128, 512], F32, tag="ps")
                    ps3 = ppqk.tile([128, 512], F32, tag="ps")
                    ps4 = ppqk.tile([128, 512], F32, tag="ps")
                    for ki in range(8):
                        st = wk8[:, C * ki + 128 * ct: C * ki + 128 * ct + 128]
                        nc.tensor.matmul(ps2[:, :], st, xk[:, ki, 0:512],
                                         start=(ki == 0), stop=(ki == 7))
                        nc.tensor.matmul(ps3[:, :], st, xk[:, ki, 512:1024],
                                         start=(ki == 0), stop=(ki == 7))
                        nc.tensor.matmul(ps4[:, 0:40], st, xk[:, ki, 1024:1064],
                                         start=(ki == 0), stop=(ki == 7))
                    for c0, cn, pst in ((0, 512, ps2), (512, 512, ps3),
                                        (1024, 40, ps4)):
                        drain(kbd[ct][0:64, c0:c0 + cn], pst[0:64, 0:cn])
                        drain(kbd[ct][64:128, R + c0:R + c0 + cn],
                              pst[64:128, 0:cn])

                def stage_a(wi):
                    tck, b, wv4 = waves[wi]
                    Ew = ep.tile([128, 1024], F32, tag="ew")
                    # merged 2-head E matmuls via block-diagonal kbd
                    for half in range(2):
                        ct = 2 * wv4 + half
                        lhs = (qT[ct][:]
                               .rearrange("p (l four) -> p l four", four=4)
                               [:, 128 * tck:128 * tck + 128, b])
                        rhs = bass.AP(kbd[ct], 4 * 128 * tck + b,
                                      [[2 * R, 128], [R, 2], [4, 138]])
                        out = bass.AP(Ew.tensor, Ew.offset + 512 * half,
                                      [[1024, 128], [160, 2], [1, 138]])
                        nc.tensor.matmul(out, lhs, rhs, start=True, stop=True)
                        nc.tensor.matmul(Ew[:, 512 * half + 298:512 * half + 320],
                                         lhs, pe2[:, 22 * ct:22 * ct + 22],
                                         start=True, stop=True)
                    # one exp over both halves (E and P regions)
                    ex = asb.tile([128, 640], BF16, tag="ex")
                    nc.scalar.activation(
                        ex[:].rearrange("p (a c) -> p a c", a=2)[:, :, 0:320],
                        Ew[:].rearrange("p (a c) -> p a c", a=2)[:, :, 0:320],
                        mybir.ActivationFunctionType.Exp,
                    )
                    edw = ed[wi]
                    nc.sync.dma_start(edw[:], ex[:, :])
                    # single batched diagonal band gather: heads at stride 160
                    bnd = bsb.tile([128, 44], BF16, tag="bnd")
                    nc.gpsimd.dma_start(
                        bnd[:].rearrange("p (h w) -> p h w", w=W),
                        bass.AP(edw, 0, [[641, 128], [160, 4], [1, W]]),
                    )
                    # numerators = band_exp * exp(P); denominator; normalize
                    exP = bass.AP(ex.tensor, ex.offset + 298,
                                  [[640, 128], [320, 2], [11, 2], [1, W]])
                    t1 = bsb.tile([128, 44], F32, tag="t1")
                    nc.vector.tensor_mul(
                        t1[:].rearrange("p (a q w) -> p a q w", a=2, q=2),
                        bnd[:].rearrange("p (a q w) -> p a q w", a=2, q=2),
                        exP,
                    )
                    den = bsb.tile([128, 4], F32, tag="den")
                    nc.vector.tensor_reduce(
                        den[:, :],
                        t1[:].rearrange("p (h w) -> p h w", w=W),
                        axis=mybir.AxisListType.X,
                        op=mybir.AluOpType.add,
                    )
                    rden = bsb.tile([128, 4], F32, tag="rden")
                    nc.vector.reciprocal(rden[:, :], den[:, :])
                    att = bsb.tile([128, 44], BF16, tag="att")
                    if BC0:
                        rbc = bass.AP(rden.tensor, rden.offset,
                                      [[4, 128], [1, 4], [0, W]])
                        nc.vector.tensor_mul(
                            att[:].rearrange("p (h w) -> p h w", w=W),
                            t1[:].rearrange("p (h w) -> p h w", w=W),
                            rbc,
                        )
                    else:
                        for hh in range(4):
                            nc.vector.tensor_scalar_mul(
                                att[:, W * hh:W * hh + W],
                                t1[:, W * hh:W * hh + W],
                                rden[:, hh:hh + 1])
                    # batched diagonal scatter into banded buffer + reload
                    afj = afd[wi % AFD_N]
                    nc.gpsimd.dma_start(
                        bass.AP(afj, 0, [[645, 128], [161, 4], [1, W]]),
                        att[:].rearrange("p (h w) -> p h w", w=W),
                    )
                    af4 = csb.tile([128, 644], BF16, tag="af4")
                    nc.sync.dma_start(af4[:, :], afj[:])
                    state[wi] = af4

                def stage_t(wi):
                    # transposes: 4 mains [128,128] + 4 tails [128,10]->[10,128]
                    af4 = state.pop(wi)
                    tps = tp.tile([128, 1024], BF16, tag="tps")
                    for hh in range(4):
                        nc.tensor.transpose(
                            tps[:, 128 * hh:128 * hh + 128],
                            af4[:, 161 * hh:161 * hh + 128], ident[:, :])
                        nc.tensor.transpose(
                            tps[0:10, 512 + 128 * hh:640 + 128 * hh],
                            af4[:, 161 * hh + 128:161 * hh + 138], ident[:, :])
                    afT = csb.tile([128, 1024], BF16, tag="afT")
                    nc.vector.tensor_copy(afT[:, 0:512], tps[:, 0:512])
                    nc.scalar.copy(afT[0:10, 512:1024], tps[0:10, 512:1024])
                    state[("afT", wi)] = afT

                def stage_b(wi):
                    tck, b, wv4 = waves[wi]
                    afT = state.pop(("afT", wi))
                    vsrc = vfull[b][1] if tck == 0 else vtail[b]
                    # AV: 4 mains (K=128) + 4 tails (K=10)
                    ops = op.tile([128, 256], F32, tag="ops")
                    for hh in range(4):
                        h = 4 * wv4 + hh
                        nc.tensor.matmul(
                            ops[:, 64 * hh:64 * hh + 64],
                            afT[:, 128 * hh:128 * hh + 128],
                            vfull[b][tck][:, 64 * h:64 * h + 64],
                            start=True, stop=False, skip_group_check=True)
                        nc.tensor.matmul(
                            ops[:, 64 * hh:64 * hh + 64],
                            afT[0:10, 512 + 128 * hh:640 + 128 * hh],
                            vsrc[0:10, 64 * h:64 * h + 64],
                            start=False, stop=True, skip_group_check=True)
                    yb = ysb.tile([128, 256], BF16, tag="yb")
                    if (wi % 2) == 0:
                        nc.vector.tensor_copy(yb[:, :], ops[:, :])
                    else:
                        nc.scalar.copy(yb[:, :], ops[:, :])
                    nc.sync.dma_start(
                        bass.AP(y_d, (512 * tck + b) * C + 256 * wv4,
                                [[4 * C, 128], [1, 256]]),
                        yb[:, :],
                    )

                # A leads by SKEW, T leads AV by 1 (drain off the PE path);
                # q/k projections for the NEXT wv4 group fill attention stalls
                qk(0)
                qk(1)
                for wi in range(min(SKEW, NW)):
                    stage_a(wi)
                stage_t(0)
                for wi in range(NW):
                    if wi + SKEW < NW:
                        stage_a(wi + SKEW)
                    g, r = divmod(wi, 8)
                    if g < 3 and r == 0:
                        qk(2 * g + 2)
                    if g < 3 and r == 2:
                        qk(2 * g + 3)
                    if wi + 1 < NW:
                        stage_t(wi + 1)
                    stage_b(wi)

    nc.compile()
    _CACHED["nc"] = nc
    return nc


def host_prep(x, Wq, Wk, Wv, pe):
    """Build per-core input maps (host-side shard + layout prep)."""
    if "host" in _CACHED:
        return _CACHED["host"]
    x = np.asarray(x, np.float32)
    xp = np.zeros((L + 2 * PAD, B, C), np.float32)
    xp[PAD:PAD + L] = x

    def w8(Wt):
        # [c_in, c_out] -> [128, (ki, c_out)]
        a = np.asarray(Wt, np.float32).reshape(8, 128, C).transpose(1, 0, 2)
        return np.ascontiguousarray(a.reshape(128, 8 * C)).astype(NPBF16)

    wq8 = w8(np.asarray(Wq, np.float32).T / 32.0)
    wk8 = w8(np.asarray(Wk, np.float32).T)
    wv8 = w8(np.asarray(Wv, np.float32).T)
    pe = np.asarray(pe, np.float32)
    # block-diagonal pe pairs: [128, (ct, 22)]
    pebd = np.zeros((128, 8, 22), np.float32)
    for ct in range(8):
        pebd[0:64, ct, 0:11] = pe[2 * ct]
        pebd[64:128, ct, 11:22] = pe[2 * ct + 1]
    pebd = np.ascontiguousarray(pebd.reshape(128, 8 * 22)).astype(NPBF16)
    ident = np.eye(128, dtype=NPBF16)
    in_maps = []
    for c in range(NCORES):
        xs = xp[LSH * c:LSH * c + LLOC].reshape(R, C)
        x8 = np.ascontiguousarray(
            xs.T.reshape(8, 128, R).transpose(1, 0, 2).reshape(128, 8 * R)
        ).astype(NPBF16)
        in_maps.append({
            "xT8": x8, "wq8": wq8, "wk8": wk8, "wv8": wv8,
            "pe2bd": pebd, "ident": ident,
        })
    _CACHED["host"] = in_maps
    return in_maps


LAST_RES = [None]


def kernel(x, Wq, Wk, Wv, pe, _want_time=False):
    nc = _build_nc()
    in_maps = host_prep(x, Wq, Wk, Wv, pe)
    kw = {}
    if _want_time:
        kw = dict(trace=True)
    res = bass_utils.run_bass_kernel_spmd(
        nc, in_maps, core_ids=list(range(NCORES)), **kw)
    LAST_RES[0] = res
    y = np.concatenate(
        [np.asarray(r["y"]).astype(np.float32) for r in res.results], axis=0)
    out = y.reshape(L, B, C)
    if _want_time:
        return out, res.exec_time_ns
    return out
